# revision 1
# baseline (speedup 1.0000x reference)
"""Trainium2 Bass kernel for nn_ConvMultiHeadAttention.

Strategy: data-parallel over batch B=8 across the 8 NeuronCores (no
collectives).  Per core (one batch element):

  1. q/k linear projections + per-head scaled-dot-product scores + masked
     softmax (bf16 matmuls, tiny).
  2. The 1x1 proj_concat conv is folded into the 3x3 value conv on the host
     (G_h = Wp_h @ Wv_h), so the conv directly produces the per-head
     projected values z[i, h, c, pos].
  3. 3x3 conv as 5 K=128 fp32r matmuls per (frame, 128-channel tile): taps
     are packed in pairs along K using column/row-shifted padded copies of
     the input frame held in SBUF partitions 0-63 / 64-127.
  4. PSUM -> SBUF bf16 eviction, then a scatter-DMA "transpose" into the
     [(h,i) on partitions, (c, pos) free] layout.
  5. Attention mix: one K=128 matmul per 512-wide slice contracting over
     (frame i, head h) at once (legal because Wp is already folded), giving
     the final output rows [o, (c, pos)].

Because softmax rows sum to 1, conv bias (Wp@bv) and proj bias bp reduce to
a constant per output channel, added on the host at the end.
"""

import os
import numpy as np

import concourse.bass as bass
import concourse.bacc as bacc
import concourse.tile as tile
import concourse.mybir as mybir
from concourse.bass_utils import run_bass_kernel_spmd

NH, DQK, DV = 8, 256, 64
B, TI, TO, H, W = 8, 16, 16, 32, 32
HW = H * W           # 1024
PW = 34              # padded row width (32 + 2)
HR = 18              # padded rows resident per half-frame (16 + 2)
N_CORES = 8

F32 = mybir.dt.float32
F32R = mybir.dt.float32r
BF16 = mybir.dt.bfloat16
I32 = mybir.dt.int32

# Tap pairing for the 5 conv matmuls (kernel indices (ky, kx) in 0..2).
# A-half = plain padded frame on partitions 0:64.
# j<3  : B-half (partitions 64:128) = frame shifted one column  -> covers kx+1
# j==3 : D-half (T2 tile)           = frame shifted one row     -> covers ky+1
# j==4 : single tap (2,2), B-half weights are zero.
_TAP_A = [(0, 0), (1, 0), (2, 0), (0, 2), (2, 2)]
_TAP_B = [(0, 1), (1, 1), (2, 1), (1, 2), None]

_GRAPH = None
LAST_RESULTS = None


def _build_graph():
    from contextlib import ExitStack

    nc = bacc.Bacc("TRN2", target_bir_lowering=False, debug=False,
                   num_devices=N_CORES)

    v_ap = nc.dram_tensor("v", [TI, DV, HW], F32, kind="ExternalInput").ap()
    q_ap = nc.dram_tensor("q", [TO, DQK], F32, kind="ExternalInput").ap()
    k_ap = nc.dram_tensor("k", [TI, DQK], F32, kind="ExternalInput").ap()
    m_ap = nc.dram_tensor("mask", [TO, TI], I32, kind="ExternalInput").ap()
    wc_ap = nc.dram_tensor("wc", [128, 2560], BF16, kind="ExternalInput").ap()
    wqk_ap = nc.dram_tensor("wqk", [64, 128, 128], BF16, kind="ExternalInput").ap()
    bqk_ap = nc.dram_tensor("bqk", [128, 32], F32, kind="ExternalInput").ap()
    id_ap = nc.dram_tensor("ident", [16, 16], F32, kind="ExternalInput").ap()
    out_ap = nc.dram_tensor("out", [TO, DV * HW], F32, kind="ExternalOutput").ap()

    AF = mybir.ActivationFunctionType
    OP = mybir.AluOpType

    with tile.TileContext(nc) as tc, ExitStack() as ctx:
        wqkp = ctx.enter_context(tc.tile_pool(name="wqkp", bufs=4))
        zevp = ctx.enter_context(tc.tile_pool(name="zevp", bufs=8))
        attp = ctx.enter_context(tc.tile_pool(name="attp", bufs=4))
        cps = ctx.enter_context(tc.tile_pool(name="cps", bufs=4, space="PSUM"))
        mps = ctx.enter_context(tc.tile_pool(name="mps", bufs=2, space="PSUM"))
        sps = ctx.enter_context(tc.tile_pool(name="sps", bufs=2, space="PSUM"))

        # persistent SBUF tensors (static allocations: no lifetime packing)
        def static(name, shape, dtype):
            return nc.alloc_sbuf_tensor(name, list(shape), dtype).ap()

        T1 = static("T1", [128, 16 * HR * PW], BF16)       # [A; B] per frame
        T2 = static("T2", [128, 16 * 16 * PW], BF16)       # [A; D] per frame
        vbf = static("vbf", [128, 16 * HW], BF16)          # frames duplicated 2x
        wc = static("wc_sb", [128, 2560], BF16)
        zT = static("zT", [128, 64 * 512], BF16)
        qk = static("qk_sb", [16, 512], F32)
        qkT = static("qkT", [128, 64], BF16)
        pqT = static("pqT", [128, 256], BF16)
        pkT = static("pkT", [128, 256], BF16)
        wflat = static("wflat", [128, 16], BF16)
        id_sb = static("id_sb", [16, 16], F32)
        bqk = static("bqk_sb", [128, 32], F32)
        mi = static("mi_sb", [16, 16], I32)
        mb = static("mb", [16, 16], F32)
        mbig = static("mbig", [16, 128], F32)
        s1 = static("s1", [16, 128], F32)
        s2 = static("s2", [16, 128], F32)
        s3 = static("s3", [16, 128], F32)
        s4 = static("s4", [16, 128], F32)
        rmax = static("rmax", [16, 8], F32)
        rsum = static("rsum", [16, 8], F32)
        rinv = static("rinv", [16, 8], F32)

        t1v = T1[:].rearrange("p (f r c) -> p f r c", r=HR, c=PW)
        t2v = T2[:].rearrange("p (f r c) -> p f r c", r=16, c=PW)
        vbfv = vbf[:].rearrange("p (f y x) -> p f y x", y=H, x=W)

        # ---------- input / constant loads ----------
        nc.sync.dma_start(qk[:, 0:256], q_ap[:, :])
        nc.sync.dma_start(qk[:, 256:512], k_ap[:, :])
        nc.sync.dma_start(mi[:], m_ap[:, :])
        nc.sync.dma_start(id_sb[:], id_ap[:, :])
        nc.sync.dma_start(bqk[:], bqk_ap[:, :])
        nc.sync.dma_start(wc[:], wc_ap[:, :])

        # zero-init the padded v staging tensors (pad cells must be 0)
        nc.vector.memset(T1[:].bitcast(F32), 0.0)
        nc.gpsimd.memset(T2[:].bitcast(F32), 0.0)

        # v frames: contiguous HBM loads (duplicated on partitions 0:64/64:128),
        # cast to bf16 once; padded T1/T2 staging is built on-chip by DVE.
        vldp = ctx.enter_context(tc.tile_pool(name="vldp", bufs=4))
        for fr in range(16):
            vld = vldp.tile([128, HW], F32, name="vld", tag="vld")
            nc.scalar.dma_start(vld[0:64, :], v_ap[fr])
            nc.scalar.dma_start(vld[64:128, :], v_ap[fr])
            nc.vector.tensor_copy(vbf[:, fr * HW:(fr + 1) * HW], vld[:])

        # ---------- scores / softmax phase ----------
        # qT / kT tiles via PE transpose: qkT cols [q-t0 | q-t1 | k-t0 | k-t1]
        for j in range(4):
            half, t = j // 2, j % 2
            ps = sps.tile([128, 16], F32, name="tps", tag="sps")
            nc.tensor.transpose(
                ps[:], qk[0:16, half * 256 + t * 128: half * 256 + (t + 1) * 128],
                id_sb[:])
            nc.vector.tensor_copy(qkT[:, j * 16:(j + 1) * 16], ps[:])

        # pqT / pkT: per m-tile of 128 (h,d)-rows, contract d' over 2 K-tiles
        for src in range(2):
            dst = pqT if src == 0 else pkT
            for m in range(16):
                ps = sps.tile([128, 16], F32, name="pps", tag="sps")
                for t in range(2):
                    wt = wqkp.tile([128, 128], BF16, name="wt", tag="wt")
                    nc.sync.dma_start(wt[:], wqk_ap[src * 32 + m * 2 + t])
                    nc.tensor.matmul(
                        ps[:], wt[:], qkT[:, (src * 2 + t) * 16:(src * 2 + t + 1) * 16],
                        start=(t == 0), stop=(t == 1))
                nc.vector.tensor_scalar_add(
                    dst[:, m * 16:(m + 1) * 16], ps[:],
                    bqk[:, src * 16 + m: src * 16 + m + 1])

        # scores[o, (h,i)]: per head contract over d (2 m-tiles)
        sc = sps.tile([16, 128], F32, name="sc", tag="sps")
        for h in range(8):
            for t in range(2):
                sl = slice((2 * h + t) * 16, (2 * h + t + 1) * 16)
                nc.tensor.matmul(sc[:, h * 16:(h + 1) * 16], pqT[:, sl], pkT[:, sl],
                                 start=(t == 0), stop=(t == 1))

        # masked softmax over i within each head block
        nc.scalar.activation(s1[:], sc[:], AF.Copy, scale=1.0 / 16.0)
        nc.vector.tensor_copy(mb[:], mi[:])
        nc.vector.tensor_scalar(mb[:], mb[:], 1.0e10, -1.0e10, OP.mult, OP.add)
        for h in range(8):
            nc.vector.tensor_copy(mbig[:, h * 16:(h + 1) * 16], mb[:])
        nc.vector.tensor_tensor(s2[:], s1[:], mbig[:], op=OP.add)
        nc.vector.reduce_max(rmax[:], s2[:].rearrange("p (h i) -> p h i", i=16),
                             axis=mybir.AxisListType.X)
        for h in range(8):
            nc.vector.tensor_scalar(s3[:, h * 16:(h + 1) * 16],
                                    s2[:, h * 16:(h + 1) * 16],
                                    rmax[:, h:h + 1], None, OP.subtract)
        nc.scalar.activation(s4[:], s3[:], AF.Exp)
        nc.vector.reduce_sum(rsum[:], s4[:].rearrange("p (h i) -> p h i", i=16),
                             axis=mybir.AxisListType.X)
        nc.vector.reciprocal(rinv[:], rsum[:])
        # write normalized weights interleaved: s3 free index = i*8 + h, so the
        # transpose below yields wflat partitions p = i*8 + h (zT layout).
        for h in range(8):
            nc.vector.tensor_scalar(s3[:, h::8],
                                    s4[:, h * 16:(h + 1) * 16],
                                    rinv[:, h:h + 1], None, OP.mult)
        wt_ps = sps.tile([128, 16], F32, name="wt_ps", tag="sps")
        nc.tensor.transpose(wt_ps[:], s3[:], id_sb[:])
        nc.vector.tensor_copy(wflat[:], wt_ps[:])

        # ---------- conv + transpose + mix, per 16-row chunk ----------
        outv = out_ap.rearrange("o (c t n) -> o c t n", t=2, n=512)

        for chunk in range(2):
            r0 = chunk * 16
            if chunk == 1:
                # row 17 of each T1 frame was data for half 0, pad for half 1
                nc.vector.memset(t1v[:, :, 17, :].bitcast(F32), 0.0)

            for fr in range(16):
                if chunk == 0:
                    # A/B rows rr 1..17  <- v rows 0..16 ; rr 0 stays zero (pad)
                    nc.vector.tensor_copy(t1v[0:64, fr, 1:18, 1:33], vbfv[0:64, fr, 0:17, :])
                    nc.vector.tensor_copy(t1v[64:128, fr, 1:18, 0:32], vbfv[64:128, fr, 0:17, :])
                    # T2: A rows rr 1..15 <- v rows 0..14 (rr 0 = pad row)
                    nc.vector.tensor_copy(t2v[0:64, fr, 1:16, 1:33], vbfv[0:64, fr, 0:15, :])
                else:
                    # A/B rows rr 0..16 <- v rows 15..31 ; rr 17 zeroed above
                    nc.vector.tensor_copy(t1v[0:64, fr, 0:17, 1:33], vbfv[0:64, fr, 15:32, :])
                    nc.vector.tensor_copy(t1v[64:128, fr, 0:17, 0:32], vbfv[64:128, fr, 15:32, :])
                    nc.vector.tensor_copy(t2v[0:64, fr, 0:16, 1:33], vbfv[0:64, fr, 15:31, :])
                # T2 D-half rows rr 0..15 <- v rows r0..r0+15 (always valid)
                nc.vector.tensor_copy(t2v[64:128, fr, 0:16, 1:33], vbfv[64:128, fr, r0:r0 + 16, :])

            for fr in range(16):
                for hp in range(4):
                    ps = cps.tile([128, 16, 32], F32, name="cpst", tag="cpst")
                    for j in range(5):
                        if j < 3:
                            rhs = t1v[:, fr, j:j + 16, 0:32]
                        elif j == 3:
                            rhs = t2v[:, fr, 0:16, 2:34]
                        else:
                            rhs = t1v[:, fr, 2:18, 2:34]
                        lhsT = wc[:, (hp * 5 + j) * 128:(hp * 5 + j + 1) * 128]
                        nc.tensor.matmul(ps[:], lhsT, rhs,
                                         start=(j == 0), stop=(j == 4))
                    z = zevp.tile([128, 512], BF16, name="zev", tag="zev")
                    nc.vector.tensor_copy(z[:], ps[:])
                    # scatter: zT partitions p = fr*8 + (2hp + h_l); free = cv*512+pos
                    dst = zT[8 * fr + 2 * hp: 8 * fr + 2 * hp + 2, :].rearrange(
                        "p (c n) -> p c n", n=512)
                    nc.sync.dma_start(dst, z[:])

            for s in range(64):
                mp = mps.tile([16, 512], F32, name="mpst", tag="mpst")
                nc.tensor.matmul(mp[:], wflat[:], zT[:, s * 512:(s + 1) * 512],
                                 start=True, stop=True)
                att = attp.tile([16, 512], F32, name="attt", tag="attt")
                nc.scalar.copy(att[:], mp[:])
                nc.scalar.dma_start(outv[:, s, chunk, :], att[:])

    nc.compile()
    return nc


def _host_consts(Wq, bq, Wk, bk, Wv, bv, Wp, bp):
    import ml_dtypes

    Wq = np.asarray(Wq, np.float32)
    Wk = np.asarray(Wk, np.float32)
    Wv = np.asarray(Wv, np.float32)
    Wp = np.asarray(Wp, np.float32)
    bq = np.asarray(bq, np.float32)
    bk = np.asarray(bk, np.float32)
    bv = np.asarray(bv, np.float32)
    bp = np.asarray(bp, np.float32)

    # fold 1x1 proj into the 3x3 conv:  G[h,co,ci,ky,kx] = sum_cm Wp[co,(h,cm)] Wv[(h,cm),ci,ky,kx]
    Wv5 = Wv.reshape(NH, DV, DV, 3, 3)
    Wp3 = Wp.reshape(DV, NH, DV)
    G = np.einsum('ohm,hmiyx->hoiyx', Wp3, Wv5).reshape(NH * DV, DV, 3, 3)

    WC = np.zeros((128, 4, 5, 128), np.float32)
    for hp in range(4):
        oc = np.arange(128) + hp * 128
        for j in range(5):
            ka, kb = _TAP_A[j], _TAP_B[j]
            WC[0:64, hp, j, :] = G[oc, :, ka[0], ka[1]].T
            if kb is not None:
                WC[64:128, hp, j, :] = G[oc, :, kb[0], kb[1]].T
    wc = np.ascontiguousarray(WC.reshape(128, 2560)).astype(ml_dtypes.bfloat16)

    wqk = np.zeros((2, 16, 2, 128, 128), np.float32)
    for i, Wmat in enumerate([Wq, Wk]):
        for m in range(16):
            for t in range(2):
                wqk[i, m, t] = Wmat[t * 128:(t + 1) * 128, m * 128:(m + 1) * 128]
    wqk = np.ascontiguousarray(wqk.reshape(64, 128, 128)).astype(ml_dtypes.bfloat16)

    bqk = np.zeros((128, 32), np.float32)
    bqk[:, 0:16] = bq.reshape(16, 128).T
    bqk[:, 16:32] = bk.reshape(16, 128).T

    ident = np.eye(16, dtype=np.float32)
    bias_total = Wp.reshape(DV, NH * DV) @ bv + bp
    return wc, wqk, bqk, ident, bias_total


def _get_graph():
    global _GRAPH
    if _GRAPH is None:
        _GRAPH = _build_graph()
    return _GRAPH


def kernel(v, k, q, prod_mask, Wq, bq, Wk, bk, Wv, bv, Wp, bp):
    global LAST_RESULTS
    nc = _get_graph()
    wc, wqk, bqk, ident, bias_total = _host_consts(Wq, bq, Wk, bk, Wv, bv, Wp, bp)

    v = np.ascontiguousarray(np.asarray(v, np.float32).reshape(B, TI, DV, HW))
    q = np.ascontiguousarray(np.asarray(q, np.float32))
    k = np.ascontiguousarray(np.asarray(k, np.float32))
    pm = np.ascontiguousarray(np.asarray(prod_mask, np.int32))

    in_maps = []
    for b in range(N_CORES):
        in_maps.append({
            "v": v[b], "q": q[b], "k": k[b], "mask": pm[b],
            "wc": wc, "wqk": wqk, "bqk": bqk, "ident": ident,
        })

    trace = bool(int(os.environ.get("KERNEL_TRACE", "0")))
    tmpdir = os.environ.get("KERNEL_TRACE_DIR") or None
    res = run_bass_kernel_spmd(nc, in_maps, core_ids=list(range(N_CORES)),
                               trace=trace, tmpdir=tmpdir)
    LAST_RESULTS = res

    out = np.stack([res.results[i]["out"] for i in range(N_CORES)])
    out = out.reshape(B, TO, DV, H, W) + bias_total[None, None, :, None, None]
    return np.ascontiguousarray(out.astype(np.float32))



# revision 11
# speedup vs baseline: 1.2248x; 1.2248x over previous
"""Trainium2 Bass kernel for nn_ConvMultiHeadAttention.

Strategy: data-parallel over batch B=8 across the 8 NeuronCores (no
collectives).  Per core (one batch element):

  1. q/k linear projections + per-head scaled-dot-product scores + masked
     softmax (bf16 matmuls, tiny).
  2. The 1x1 proj_concat conv is folded into the 3x3 value conv on the host
     (G_h = Wp_h @ Wv_h), so the conv directly produces the per-head
     projected values z[i, h, c, pos].
  3. 3x3 conv as 5 K=128 matmuls per (frame, 128-channel tile): taps
     are packed in pairs along K using column/row-shifted padded copies of
     the input frame held in SBUF partitions 0-63 / 64-127.
  4. PSUM -> SBUF bf16 eviction into a per-frame [128, 4*512] tile, then ONE
     scatter-DMA "transpose" per frame into the [(i,h) on partitions,
     (c, pos) free] layout (alternating sync/scalar DMA rings).
  5. Attention mix: 4x column-tiled K=128 matmuls (tile_position) contracting
     over (frame i, head h), 4 slices concurrently in the PE array, evicted
     into one big SBUF tile and written with ONE output DMA per chunk (bf16).

v2 changes vs baseline: all weights preloaded in single DMAs, v loaded via
gpsimd cast-DMAs straight to bf16, 16 scatter DMAs instead of 128, 2 output
DMAs instead of 128 -- the DMA rings and PE stay busy, HAM stays warm.
"""

import os
import numpy as np

import concourse.bass as bass
import concourse.bacc as bacc
import concourse.tile as tile
import concourse.mybir as mybir
from concourse.bass_utils import run_bass_kernel_spmd

NH, DQK, DV = 8, 256, 64
B, TI, TO, H, W = 8, 16, 16, 32, 32
HW = H * W           # 1024
PW = 34              # padded row width (32 + 2)
HR = 18              # padded rows resident per half-frame (16 + 2)
N_CORES = 8

F32 = mybir.dt.float32
BF16 = mybir.dt.bfloat16
I32 = mybir.dt.int32

# Tap pairing for the 5 conv matmuls (kernel indices (ky, kx) in 0..2).
# A-half = plain padded frame on partitions 0:64.
# j<3  : B-half (partitions 64:128) = frame shifted one column  -> covers kx+1
# j==3 : D-half (T2 tile)           = frame shifted one row     -> covers ky+1
# j==4 : single tap (2,2), B-half weights are zero.
_TAP_A = [(0, 0), (1, 0), (2, 0), (0, 2), (2, 2)]
_TAP_B = [(0, 1), (1, 1), (2, 1), (1, 2), None]

_GRAPH = None
LAST_RESULTS = None


def _build_graph():
    from contextlib import ExitStack

    nc = bacc.Bacc("TRN2", target_bir_lowering=False, debug=False,
                   num_devices=N_CORES)

    v_ap = nc.dram_tensor("v", [TI, DV, HW], F32, kind="ExternalInput").ap()
    q_ap = nc.dram_tensor("q", [TO, DQK], F32, kind="ExternalInput").ap()
    k_ap = nc.dram_tensor("k", [TI, DQK], F32, kind="ExternalInput").ap()
    m_ap = nc.dram_tensor("mask", [TO, TI], I32, kind="ExternalInput").ap()
    wc_ap = nc.dram_tensor("wc", [128, 2560], BF16, kind="ExternalInput").ap()
    wqk_ap = nc.dram_tensor("wqk", [128, 8192], BF16, kind="ExternalInput").ap()
    bqk_ap = nc.dram_tensor("bqk", [128, 32], F32, kind="ExternalInput").ap()
    id_ap = nc.dram_tensor("ident", [16, 16], F32, kind="ExternalInput").ap()
    out_ap = nc.dram_tensor("out", [TO, DV * HW], BF16, kind="ExternalOutput").ap()

    AF = mybir.ActivationFunctionType
    OP = mybir.AluOpType

    with tile.TileContext(nc) as tc, ExitStack() as ctx:
        zevp = ctx.enter_context(tc.tile_pool(name="zevp", bufs=4))
        cps = ctx.enter_context(tc.tile_pool(name="cps", bufs=4, space="PSUM"))
        mps = ctx.enter_context(tc.tile_pool(name="mps", bufs=2, space="PSUM"))
        sps = ctx.enter_context(tc.tile_pool(name="sps", bufs=2, space="PSUM"))

        # persistent SBUF tensors (static allocations: no lifetime packing)
        def static(name, shape, dtype):
            return nc.alloc_sbuf_tensor(name, list(shape), dtype).ap()

        T1 = static("T1", [128, 16 * HR * PW], BF16)       # [A; B] per frame
        T2 = static("T2", [128, 16 * 16 * PW], BF16)       # [A; D] per frame
        vbf = static("vbf", [128, 16 * HW], BF16)          # frames duplicated 2x
        wc = static("wc_sb", [128, 2560], BF16)
        wqk = static("wqk_sb", [128, 8192], BF16)
        zT = static("zT", [128, 64 * 512], BF16)
        att = static("att_sb", [128, 16 * 512], BF16)
        qk = static("qk_sb", [16, 512], F32)
        qkT = static("qkT", [128, 64], BF16)
        pqT = static("pqT", [128, 256], BF16)
        pkT = static("pkT", [128, 256], BF16)
        wflat = static("wflat", [128, 32], BF16)
        id_sb = static("id_sb", [16, 16], F32)
        bqk = static("bqk_sb", [128, 32], F32)
        mi = static("mi_sb", [16, 16], I32)
        mb = static("mb", [16, 16], F32)
        mbig = static("mbig", [16, 128], F32)
        s1 = static("s1", [16, 128], F32)
        s2 = static("s2", [16, 128], F32)
        s3 = static("s3", [16, 128], F32)
        s4 = static("s4", [16, 128], F32)
        rmax = static("rmax", [16, 8], F32)
        rsum = static("rsum", [16, 8], F32)
        rinv = static("rinv", [16, 8], F32)

        t1v = T1[:].rearrange("p (f r c) -> p f r c", r=HR, c=PW)
        t2v = T2[:].rearrange("p (f r c) -> p f r c", r=16, c=PW)
        vbfv = vbf[:].rearrange("p (f y x) -> p f y x", y=H, x=W)

        # ---------- input / constant loads ----------
        # sync ring: small qk-phase inputs first, then the big weight blocks
        nc.sync.dma_start(qk[:, 0:256], q_ap[:, :])
        nc.sync.dma_start(qk[:, 256:512], k_ap[:, :])
        nc.sync.dma_start(id_sb[:], id_ap[:, :])
        nc.sync.dma_start(mi[:], m_ap[:, :])
        nc.sync.dma_start(bqk[:], bqk_ap[:, :])
        nc.sync.dma_start(wqk[:], wqk_ap[:, :])
        nc.sync.dma_start(wc[:], wc_ap[:, :])

        # v frames: gpsimd (SWDGE) DMAs cast f32 -> bf16 on the fly and land
        # the frames duplicated on partitions 0:64 / 64:128.  Split into 4
        # DMAs so staging of frames 0-7 can start while 8-15 still load.
        vbf_h0 = vbf[0:64].rearrange("p (f n) -> p f n", n=HW)
        vbf_h1 = vbf[64:128].rearrange("p (f n) -> p f n", n=HW)
        for lo, hi in ((0, 8), (8, 16)):
            src = v_ap[lo:hi].rearrange("f c n -> c f n")
            nc.gpsimd.dma_start(vbf_h0[:, lo:hi, :], src)
            nc.gpsimd.dma_start(vbf_h1[:, lo:hi, :], src)

        # zero-init the padded v staging tensors (pad cells must be 0)
        nc.vector.memset(T1[:].bitcast(F32), 0.0)
        nc.vector.memset(T2[:].bitcast(F32), 0.0)

        # ---------- scores / softmax phase ----------
        # qT / kT tiles via PE transpose: qkT cols [q-t0 | q-t1 | k-t0 | k-t1]
        for j in range(4):
            half, t = j // 2, j % 2
            ps = sps.tile([128, 16], F32, name="tps", tag="sps")
            nc.tensor.transpose(
                ps[:], qk[0:16, half * 256 + t * 128: half * 256 + (t + 1) * 128],
                id_sb[:])
            nc.vector.tensor_copy(qkT[:, j * 16:(j + 1) * 16], ps[:])

        # pqT / pkT: per m-tile of 128 (h,d)-rows, contract d' over 2 K-tiles
        for src in range(2):
            dst = pqT if src == 0 else pkT
            for m in range(16):
                ps = sps.tile([128, 16], F32, name="pps", tag="sps")
                for t in range(2):
                    ti = (src * 32 + m * 2 + t) * 128
                    nc.tensor.matmul(
                        ps[:], wqk[:, ti:ti + 128],
                        qkT[:, (src * 2 + t) * 16:(src * 2 + t + 1) * 16],
                        start=(t == 0), stop=(t == 1))
                nc.vector.tensor_scalar_add(
                    dst[:, m * 16:(m + 1) * 16], ps[:],
                    bqk[:, src * 16 + m: src * 16 + m + 1])

        # scores[o, (h,i)]: per head contract over d (2 m-tiles)
        sc = sps.tile([16, 128], F32, name="sc", tag="sps")
        for h in range(8):
            for t in range(2):
                sl = slice((2 * h + t) * 16, (2 * h + t + 1) * 16)
                nc.tensor.matmul(sc[:, h * 16:(h + 1) * 16], pqT[:, sl], pkT[:, sl],
                                 start=(t == 0), stop=(t == 1))

        # masked softmax over i within each head block
        nc.scalar.activation(s1[:], sc[:], AF.Copy, scale=1.0 / 16.0)
        nc.vector.tensor_copy(mb[:], mi[:])
        nc.vector.tensor_scalar(mb[:], mb[:], 1.0e10, -1.0e10, OP.mult, OP.add)
        for h in range(8):
            nc.vector.tensor_copy(mbig[:, h * 16:(h + 1) * 16], mb[:])
        nc.vector.tensor_tensor(s2[:], s1[:], mbig[:], op=OP.add)
        nc.vector.reduce_max(rmax[:], s2[:].rearrange("p (h i) -> p h i", i=16),
                             axis=mybir.AxisListType.X)
        for h in range(8):
            nc.vector.tensor_scalar(s3[:, h * 16:(h + 1) * 16],
                                    s2[:, h * 16:(h + 1) * 16],
                                    rmax[:, h:h + 1], None, OP.subtract)
        nc.scalar.activation(s4[:], s3[:], AF.Exp)
        nc.vector.reduce_sum(rsum[:], s4[:].rearrange("p (h i) -> p h i", i=16),
                             axis=mybir.AxisListType.X)
        nc.vector.reciprocal(rinv[:], rsum[:])
        # write normalized weights interleaved: s3 free index = i*8 + h, so the
        # transpose below yields wflat partitions p = i*8 + h (zT layout).
        for h in range(8):
            nc.vector.tensor_scalar(s3[:, h::8],
                                    s4[:, h * 16:(h + 1) * 16],
                                    rinv[:, h:h + 1], None, OP.mult)
        # wflat cols 16:32 stay zero so the 32-wide col-tiled mix matmuls
        # write fully-defined PSUM ranges (rows 16:32 produce zeros).
        nc.vector.memset(wflat[:].bitcast(F32), 0.0)
        wt_ps = sps.tile([128, 16], F32, name="wt_ps", tag="sps")
        nc.tensor.transpose(wt_ps[:], s3[:], id_sb[:])
        nc.vector.tensor_copy(wflat[:, 0:16], wt_ps[:])

        # ---------- conv + transpose + mix, per 16-row chunk ----------
        attv = att[:].rearrange("p (g n) -> p g n", n=512)

        def zT8(fr, hp):
            # [2 partitions (h2), 64 (c), 512 (n)] view of zT rows for (fr, hp)
            return zT[8 * fr + 2 * hp: 8 * fr + 2 * hp + 2].rearrange(
                "p (c n) -> p c n", n=512)

        for chunk in range(2):
            r0 = chunk * 16
            if chunk == 1:
                # row 17 of each T1 frame was data for half 0, pad for half 1
                nc.vector.memset(t1v[:, :, 17, :].bitcast(F32), 0.0)

            for fr in range(16):
                # staging: T1 (A/B halves) on vector, T2 (A/D halves) on gpsimd
                if chunk == 0:
                    # A/B rows rr 1..17  <- v rows 0..16 ; rr 0 stays zero (pad)
                    nc.vector.tensor_copy(t1v[0:64, fr, 1:18, 1:33], vbfv[0:64, fr, 0:17, :])
                    nc.vector.tensor_copy(t1v[64:128, fr, 1:18, 0:32], vbfv[64:128, fr, 0:17, :])
                    # T2: A rows rr 1..15 <- v rows 0..14 (rr 0 = pad row)
                    nc.gpsimd.tensor_copy(t2v[0:64, fr, 1:16, 1:33], vbfv[0:64, fr, 0:15, :])
                else:
                    # A/B rows rr 0..16 <- v rows 15..31 ; rr 17 zeroed above
                    nc.vector.tensor_copy(t1v[0:64, fr, 0:17, 1:33], vbfv[0:64, fr, 15:32, :])
                    nc.vector.tensor_copy(t1v[64:128, fr, 0:17, 0:32], vbfv[64:128, fr, 15:32, :])
                    nc.gpsimd.tensor_copy(t2v[0:64, fr, 0:16, 1:33], vbfv[0:64, fr, 15:31, :])
                # T2 D-half rows rr 0..15 <- v rows r0..r0+15 (always valid)
                nc.gpsimd.tensor_copy(t2v[64:128, fr, 0:16, 1:33], vbfv[64:128, fr, r0:r0 + 16, :])

            for fr in range(16):
                for hp in range(4):
                    ps = cps.tile([128, 16, 32], F32, name="cpst", tag="cpst")
                    for j in range(5):
                        if j < 3:
                            rhs = t1v[:, fr, j:j + 16, 0:32]
                        elif j == 3:
                            rhs = t2v[:, fr, 0:16, 2:34]
                        else:
                            rhs = t1v[:, fr, 2:18, 2:34]
                        lhsT = wc[:, (hp * 5 + j) * 128:(hp * 5 + j + 1) * 128]
                        nc.tensor.matmul(ps[:], lhsT, rhs,
                                         start=(j == 0), stop=(j == 4))
                    # evict PSUM -> bf16 tile (alternate DVE/ACT)
                    z = zevp.tile([128, 512], BF16, name="zev", tag="zev")
                    if hp % 2 == 0:
                        nc.vector.tensor_copy(z[:], ps[:])
                    else:
                        nc.scalar.copy(z[:], ps[:])
                    # scatter: zT partitions p = fr*8 + (2hp + h_l); free = c*512+pos
                    eng = nc.sync if (fr * 4 + hp) % 2 == 0 else nc.scalar
                    eng.dma_start(zT8(fr, hp), z[:])

            # mix: 4 column-tiled K=128 matmuls at once (slices s=4g+jj),
            # output rows o live at PSUM partitions 32*jj .. 32*jj+16
            for g in range(16):
                mp = mps.tile([128, 512], F32, name="mpst", tag="mpst")
                for jj in range(4):
                    s = g * 4 + jj
                    nc.tensor.matmul(mp[32 * jj:32 * jj + 32, :], wflat[:],
                                     zT[:, s * 512:(s + 1) * 512],
                                     start=True, stop=True,
                                     tile_position=(0, 32 * jj))
                nc.vector.tensor_copy(attv[:, g, :], mp[:])

            # 4 output DMAs per chunk (bf16, one per col-group); host casts
            # to f32 and adds the bias.
            outv = out_ap.rearrange("o (g jj pos) -> jj o g pos", jj=4, pos=1024)
            for jj in range(4):
                src = att[32 * jj:32 * jj + 16].rearrange("p (g n) -> p g n", n=512)
                dst = outv[jj, :, :, chunk * 512:(chunk + 1) * 512]
                eng = nc.scalar if jj % 2 == 0 else nc.sync
                eng.dma_start(dst, src)

    nc.compile()
    return nc


def _host_consts(Wq, bq, Wk, bk, Wv, bv, Wp, bp):
    import ml_dtypes

    Wq = np.asarray(Wq, np.float32)
    Wk = np.asarray(Wk, np.float32)
    Wv = np.asarray(Wv, np.float32)
    Wp = np.asarray(Wp, np.float32)
    bq = np.asarray(bq, np.float32)
    bk = np.asarray(bk, np.float32)
    bv = np.asarray(bv, np.float32)
    bp = np.asarray(bp, np.float32)

    # fold 1x1 proj into the 3x3 conv:  G[h,co,ci,ky,kx] = sum_cm Wp[co,(h,cm)] Wv[(h,cm),ci,ky,kx]
    Wv5 = Wv.reshape(NH, DV, DV, 3, 3)
    Wp3 = Wp.reshape(DV, NH, DV)
    G = np.einsum('ohm,hmiyx->hoiyx', Wp3, Wv5).reshape(NH * DV, DV, 3, 3)

    WC = np.zeros((128, 4, 5, 128), np.float32)
    for hp in range(4):
        oc = np.arange(128) + hp * 128
        for j in range(5):
            ka, kb = _TAP_A[j], _TAP_B[j]
            WC[0:64, hp, j, :] = G[oc, :, ka[0], ka[1]].T
            if kb is not None:
                WC[64:128, hp, j, :] = G[oc, :, kb[0], kb[1]].T
    wc = np.ascontiguousarray(WC.reshape(128, 2560)).astype(ml_dtypes.bfloat16)

    wqk = np.zeros((2, 16, 2, 128, 128), np.float32)
    for i, Wmat in enumerate([Wq, Wk]):
        for m in range(16):
            for t in range(2):
                wqk[i, m, t] = Wmat[t * 128:(t + 1) * 128, m * 128:(m + 1) * 128]
    # flatten to [K=128 partitions, tile*128 + m-col] for the single preload
    wqk = np.ascontiguousarray(
        wqk.reshape(64, 128, 128).transpose(1, 0, 2).reshape(128, 8192)
    ).astype(ml_dtypes.bfloat16)

    bqk = np.zeros((128, 32), np.float32)
    bqk[:, 0:16] = bq.reshape(16, 128).T
    bqk[:, 16:32] = bk.reshape(16, 128).T

    ident = np.eye(16, dtype=np.float32)
    bias_total = Wp.reshape(DV, NH * DV) @ bv + bp
    return wc, wqk, bqk, ident, bias_total


def _get_graph():
    global _GRAPH
    if _GRAPH is None:
        _GRAPH = _build_graph()
    return _GRAPH


def kernel(v, k, q, prod_mask, Wq, bq, Wk, bk, Wv, bv, Wp, bp):
    global LAST_RESULTS
    nc = _get_graph()
    wc, wqk, bqk, ident, bias_total = _host_consts(Wq, bq, Wk, bk, Wv, bv, Wp, bp)

    v = np.ascontiguousarray(np.asarray(v, np.float32).reshape(B, TI, DV, HW))
    q = np.ascontiguousarray(np.asarray(q, np.float32))
    k = np.ascontiguousarray(np.asarray(k, np.float32))
    pm = np.ascontiguousarray(np.asarray(prod_mask, np.int32))

    in_maps = []
    for b in range(N_CORES):
        in_maps.append({
            "v": v[b], "q": q[b], "k": k[b], "mask": pm[b],
            "wc": wc, "wqk": wqk, "bqk": bqk, "ident": ident,
        })

    trace = bool(int(os.environ.get("KERNEL_TRACE", "0")))
    tmpdir = os.environ.get("KERNEL_TRACE_DIR") or None
    res = run_bass_kernel_spmd(nc, in_maps, core_ids=list(range(N_CORES)),
                               trace=trace, tmpdir=tmpdir)
    LAST_RESULTS = res

    out = np.stack([np.asarray(res.results[i]["out"]) for i in range(N_CORES)])
    out = out.astype(np.float32).reshape(B, TO, DV, H, W)
    out = out + bias_total[None, None, :, None, None]
    return np.ascontiguousarray(out)


# revision 20
# speedup vs baseline: 1.4458x; 1.1805x over previous
"""Trainium2 Bass kernel for nn_ConvMultiHeadAttention.

Strategy: data-parallel over batch B=8 across the 8 NeuronCores (no
collectives).  Per core (one batch element):

  1. q/k linear projections + per-head scaled-dot-product scores + masked
     softmax (bf16 matmuls, tiny).
  2. The 1x1 proj_concat conv is folded into the 3x3 value conv on the host
     (G_h = Wp_h @ Wv_h), so the conv directly produces the per-head
     projected values z[i, h, c, pos].
  3. 3x3 conv as 5 K=128 matmuls per (frame, 128-channel tile): taps
     are packed in pairs along K using column/row-shifted padded copies of
     the input frame held in SBUF partitions 0-63 / 64-127.
  4. PSUM -> SBUF bf16 eviction into a per-frame [128, 4*512] tile, then ONE
     scatter-DMA "transpose" per frame into the [(i,h) on partitions,
     (c, pos) free] layout (alternating sync/scalar DMA rings).
  5. Attention mix: 4x column-tiled K=128 matmuls (tile_position) contracting
     over (frame i, head h), 4 slices concurrently in the PE array, evicted
     into one big SBUF tile and written with ONE output DMA per chunk (bf16).

v2 changes vs baseline: all weights preloaded in single DMAs, v loaded via
gpsimd cast-DMAs straight to bf16, 16 scatter DMAs instead of 128, 2 output
DMAs instead of 128 -- the DMA rings and PE stay busy, HAM stays warm.
"""

import os
import numpy as np

import concourse.bass as bass
import concourse.bacc as bacc
import concourse.tile as tile
import concourse.mybir as mybir
from concourse.bass_utils import run_bass_kernel_spmd

NH, DQK, DV = 8, 256, 64
B, TI, TO, H, W = 8, 16, 16, 32, 32
HW = H * W           # 1024
PW = 34              # padded row width (32 + 2)
HR = 18              # padded rows resident per half-frame (16 + 2)
N_CORES = 8

F32 = mybir.dt.float32
BF16 = mybir.dt.bfloat16
I32 = mybir.dt.int32

# Tap pairing for the 5 conv matmuls (kernel indices (ky, kx) in 0..2).
# A-half = plain padded frame on partitions 0:64.
# j<3  : B-half (partitions 64:128) = frame shifted one column  -> covers kx+1
# j==3 : D-half (T2 tile)           = frame shifted one row     -> covers ky+1
# j==4 : single tap (2,2), B-half weights are zero.
_TAP_A = [(0, 0), (1, 0), (2, 0), (0, 2), (2, 2)]
_TAP_B = [(0, 1), (1, 1), (2, 1), (1, 2), None]

_GRAPH = None
LAST_RESULTS = None


def _build_graph():
    from contextlib import ExitStack

    nc = bacc.Bacc("TRN2", target_bir_lowering=False, debug=False,
                   num_devices=N_CORES)

    v_ap = nc.dram_tensor("v", [TI, DV, HW], F32, kind="ExternalInput").ap()
    q_ap = nc.dram_tensor("q", [TO, DQK], F32, kind="ExternalInput").ap()
    k_ap = nc.dram_tensor("k", [TI, DQK], F32, kind="ExternalInput").ap()
    m_ap = nc.dram_tensor("mask", [TO, TI], I32, kind="ExternalInput").ap()
    wc_ap = nc.dram_tensor("wc", [128, 2560], BF16, kind="ExternalInput").ap()
    wqk_ap = nc.dram_tensor("wqk", [128, 8192], BF16, kind="ExternalInput").ap()
    bqk_ap = nc.dram_tensor("bqk", [128, 32], F32, kind="ExternalInput").ap()
    id_ap = nc.dram_tensor("ident", [16, 16], F32, kind="ExternalInput").ap()
    out_ap = nc.dram_tensor("out", [TO, DV * HW], BF16, kind="ExternalOutput").ap()

    AF = mybir.ActivationFunctionType
    OP = mybir.AluOpType

    with tile.TileContext(nc) as tc, ExitStack() as ctx:
        zevp = ctx.enter_context(tc.tile_pool(name="zevp", bufs=8))
        cps = ctx.enter_context(tc.tile_pool(name="cps", bufs=6, space="PSUM"))
        sps = ctx.enter_context(tc.tile_pool(name="sps", bufs=2, space="PSUM"))

        # persistent SBUF tensors (static allocations: no lifetime packing)
        def static(name, shape, dtype):
            return nc.alloc_sbuf_tensor(name, list(shape), dtype).ap()

        T1 = static("T1", [128, 16 * HR * PW], BF16)       # [A; B] per frame
        T2 = static("T2", [128, 16 * 16 * PW], BF16)       # [A; D] per frame
        vbf = static("vbf", [128, 16 * HW], BF16)          # frames duplicated 2x
        wc = static("wc_sb", [128, 2560], BF16)
        wqk = static("wqk_sb", [128, 8192], BF16)
        zT = static("zT", [128, 64 * 512], BF16)
        att = static("att_sb", [128, 16 * 512], BF16)
        qk = static("qk_sb", [16, 512], F32)
        qkT = static("qkT", [128, 64], BF16)
        pqT = static("pqT", [128, 256], BF16)
        pkT = static("pkT", [128, 256], BF16)
        wflat = static("wflat", [128, 32], BF16)
        id_sb = static("id_sb", [16, 16], F32)
        bqk = static("bqk_sb", [128, 32], F32)
        mi = static("mi_sb", [16, 16], I32)
        mb = static("mb", [16, 16], F32)
        mbig = static("mbig", [16, 128], F32)
        s1 = static("s1", [16, 128], F32)
        s2 = static("s2", [16, 128], F32)
        s3 = static("s3", [16, 128], F32)
        s4 = static("s4", [16, 128], F32)
        rmax = static("rmax", [16, 8], F32)
        rsum = static("rsum", [16, 8], F32)
        rinv = static("rinv", [16, 8], F32)

        t1v = T1[:].rearrange("p (f r c) -> p f r c", r=HR, c=PW)
        t2v = T2[:].rearrange("p (f r c) -> p f r c", r=16, c=PW)
        vbfv = vbf[:].rearrange("p (f y x) -> p f y x", y=H, x=W)

        # ---------- input / constant loads ----------
        # sync ring: small qk-phase inputs first, then the big weight blocks
        nc.sync.dma_start(qk[:, 0:256], q_ap[:, :])
        nc.sync.dma_start(qk[:, 256:512], k_ap[:, :])
        nc.sync.dma_start(id_sb[:], id_ap[:, :])
        nc.sync.dma_start(mi[:], m_ap[:, :])
        nc.sync.dma_start(bqk[:], bqk_ap[:, :])
        nc.sync.dma_start(wqk[:], wqk_ap[:, :])
        nc.sync.dma_start(wc[:], wc_ap[:, :])

        # v frames: gpsimd (SWDGE) DMAs cast f32 -> bf16 on the fly and land
        # the frames duplicated on partitions 0:64 / 64:128.  Split into 4
        # DMAs so staging of frames 0-7 can start while 8-15 still load.
        vbf_h0 = vbf[0:64].rearrange("p (f n) -> p f n", n=HW)
        vbf_h1 = vbf[64:128].rearrange("p (f n) -> p f n", n=HW)
        for lo, hi in ((0, 8), (8, 16)):
            src = v_ap[lo:hi].rearrange("f c n -> c f n")
            nc.gpsimd.dma_start(vbf_h0[:, lo:hi, :], src)
            nc.gpsimd.dma_start(vbf_h1[:, lo:hi, :], src)

        # zero-init the padded v staging tensors (pad cells must be 0)
        nc.vector.memset(T1[:].bitcast(F32), 0.0)
        nc.vector.memset(T2[:].bitcast(F32), 0.0)

        # ---------- scores / softmax phase ----------
        # All PSUM tiles in the sps pool share one tag/shape (a full bank)
        # so the pool costs exactly 2 banks; small users slice it.
        def sps_tile():
            return sps.tile([128, 512], F32, name="spst", tag="spst")

        # qT / kT tiles via PE transpose: qkT cols [q-t0 | q-t1 | k-t0 | k-t1]
        for j in range(4):
            half, t = j // 2, j % 2
            ps = sps_tile()
            nc.tensor.transpose(
                ps[:, 0:16], qk[0:16, half * 256 + t * 128: half * 256 + (t + 1) * 128],
                id_sb[:])
            nc.vector.tensor_copy(qkT[:, j * 16:(j + 1) * 16], ps[:, 0:16])

        # pqT / pkT: per m-tile of 128 (h,d)-rows, contract d' over 2 K-tiles
        for src in range(2):
            dst = pqT if src == 0 else pkT
            for m in range(16):
                ps = sps_tile()
                for t in range(2):
                    ti = (src * 32 + m * 2 + t) * 128
                    nc.tensor.matmul(
                        ps[:, 0:16], wqk[:, ti:ti + 128],
                        qkT[:, (src * 2 + t) * 16:(src * 2 + t + 1) * 16],
                        start=(t == 0), stop=(t == 1))
                nc.vector.tensor_scalar_add(
                    dst[:, m * 16:(m + 1) * 16], ps[:, 0:16],
                    bqk[:, src * 16 + m: src * 16 + m + 1])

        # scores[o, (h,i)]: per head contract over d (2 m-tiles)
        sc_t = sps_tile()
        sc = sc_t[0:16, 0:128]
        for h in range(8):
            for t in range(2):
                sl = slice((2 * h + t) * 16, (2 * h + t + 1) * 16)
                nc.tensor.matmul(sc[:, h * 16:(h + 1) * 16], pqT[:, sl], pkT[:, sl],
                                 start=(t == 0), stop=(t == 1))

        # masked softmax over i within each head block
        nc.scalar.activation(s1[:], sc, AF.Copy, scale=1.0 / 16.0)
        nc.vector.tensor_copy(mb[:], mi[:])
        nc.vector.tensor_scalar(mb[:], mb[:], 1.0e10, -1.0e10, OP.mult, OP.add)
        for h in range(8):
            nc.vector.tensor_copy(mbig[:, h * 16:(h + 1) * 16], mb[:])
        nc.vector.tensor_tensor(s2[:], s1[:], mbig[:], op=OP.add)
        nc.vector.reduce_max(rmax[:], s2[:].rearrange("p (h i) -> p h i", i=16),
                             axis=mybir.AxisListType.X)
        for h in range(8):
            nc.vector.tensor_scalar(s3[:, h * 16:(h + 1) * 16],
                                    s2[:, h * 16:(h + 1) * 16],
                                    rmax[:, h:h + 1], None, OP.subtract)
        nc.scalar.activation(s4[:], s3[:], AF.Exp)
        nc.vector.reduce_sum(rsum[:], s4[:].rearrange("p (h i) -> p h i", i=16),
                             axis=mybir.AxisListType.X)
        nc.vector.reciprocal(rinv[:], rsum[:])
        # write normalized weights interleaved: s3 free index = i*8 + h, so the
        # transpose below yields wflat partitions p = i*8 + h (zT layout).
        for h in range(8):
            nc.vector.tensor_scalar(s3[:, h::8],
                                    s4[:, h * 16:(h + 1) * 16],
                                    rinv[:, h:h + 1], None, OP.mult)
        # wflat cols 16:32 stay zero so the 32-wide col-tiled mix matmuls
        # write fully-defined PSUM ranges (rows 16:32 produce zeros).
        nc.vector.memset(wflat[:].bitcast(F32), 0.0)
        wt_ps = sps_tile()
        nc.tensor.transpose(wt_ps[:, 0:16], s3[:], id_sb[:])
        nc.vector.tensor_copy(wflat[:, 0:16], wt_ps[:, 0:16])

        # ---------- conv + transpose + mix, per 16-row chunk ----------
        attv = att[:].rearrange("p (g n) -> p g n", n=512)

        def zT8(fr, hp):
            # [2 partitions (h2), 64 (c), 512 (n)] view of zT rows for (fr, hp)
            return zT[8 * fr + 2 * hp: 8 * fr + 2 * hp + 2].rearrange(
                "p (c n) -> p c n", n=512)

        for chunk in range(2):
            r0 = chunk * 16
            if chunk == 1:
                # row 17 of each T1 frame was data for half 0, pad for half 1
                nc.vector.memset(t1v[:, :, 17, :].bitcast(F32), 0.0)

            for fr in range(16):
                # staging: T1 (A/B halves) on vector, T2 (A/D halves) on gpsimd
                if chunk == 0:
                    # A/B rows rr 1..17  <- v rows 0..16 ; rr 0 stays zero (pad)
                    nc.vector.tensor_copy(t1v[0:64, fr, 1:18, 1:33], vbfv[0:64, fr, 0:17, :])
                    nc.vector.tensor_copy(t1v[64:128, fr, 1:18, 0:32], vbfv[64:128, fr, 0:17, :])
                    # T2: A rows rr 1..15 <- v rows 0..14 (rr 0 = pad row)
                    nc.vector.tensor_copy(t2v[0:64, fr, 1:16, 1:33], vbfv[0:64, fr, 0:15, :])
                else:
                    # A/B rows rr 0..16 <- v rows 15..31 ; rr 17 zeroed above
                    nc.vector.tensor_copy(t1v[0:64, fr, 0:17, 1:33], vbfv[0:64, fr, 15:32, :])
                    nc.vector.tensor_copy(t1v[64:128, fr, 0:17, 0:32], vbfv[64:128, fr, 15:32, :])
                    nc.vector.tensor_copy(t2v[0:64, fr, 0:16, 1:33], vbfv[0:64, fr, 15:31, :])
                # T2 D-half rows rr 0..15 <- v rows r0..r0+15 (always valid)
                nc.vector.tensor_copy(t2v[64:128, fr, 0:16, 1:33], vbfv[64:128, fr, r0:r0 + 16, :])

            for fr in range(16):
                for hp in range(4):
                    ps = cps.tile([128, 16, 32], F32, name="cpst", tag="cpst")
                    for j in range(5):
                        if j < 3:
                            rhs = t1v[:, fr, j:j + 16, 0:32]
                        elif j == 3:
                            rhs = t2v[:, fr, 0:16, 2:34]
                        else:
                            rhs = t1v[:, fr, 2:18, 2:34]
                        lhsT = wc[:, (hp * 5 + j) * 128:(hp * 5 + j + 1) * 128]
                        nc.tensor.matmul(ps[:], lhsT, rhs,
                                         start=(j == 0), stop=(j == 4))
                    # evict PSUM -> bf16 tile on the scalar engine (DVE is
                    # busy with staging + mix evictions)
                    z = zevp.tile([128, 512], BF16, name="zev", tag="zev")
                    nc.scalar.copy(z[:], ps[:])
                    # scatter: zT partitions p = fr*8 + (2hp + h_l); free =
                    # c*512+pos -- alternate between the two HWDGE rings
                    eng = (nc.sync, nc.scalar)[(fr * 4 + hp) % 2]
                    eng.dma_start(zT8(fr, hp), z[:])

            # mix: 4 column-tiled K=128 matmuls at once (slices s=4g+jj),
            # output rows o live at PSUM partitions 32*jj .. 32*jj+16
            for g in range(16):
                mp = sps_tile()
                for jj in range(4):
                    s = g * 4 + jj
                    nc.tensor.matmul(mp[32 * jj:32 * jj + 32, :], wflat[:],
                                     zT[:, s * 512:(s + 1) * 512],
                                     start=True, stop=True,
                                     tile_position=(0, 32 * jj))
                nc.vector.tensor_copy(attv[:, g, :], mp[:])

            # 4 output DMAs per chunk (bf16, one per col-group); host casts
            # to f32 and adds the bias.
            outv = out_ap.rearrange("o (g jj pos) -> jj o g pos", jj=4, pos=1024)
            for jj in range(4):
                src = att[32 * jj:32 * jj + 16].rearrange("p (g n) -> p g n", n=512)
                dst = outv[jj, :, :, chunk * 512:(chunk + 1) * 512]
                eng = nc.scalar if jj % 2 == 0 else nc.sync
                eng.dma_start(dst, src)

    nc.compile()
    return nc


def _host_consts(Wq, bq, Wk, bk, Wv, bv, Wp, bp):
    import ml_dtypes

    Wq = np.asarray(Wq, np.float32)
    Wk = np.asarray(Wk, np.float32)
    Wv = np.asarray(Wv, np.float32)
    Wp = np.asarray(Wp, np.float32)
    bq = np.asarray(bq, np.float32)
    bk = np.asarray(bk, np.float32)
    bv = np.asarray(bv, np.float32)
    bp = np.asarray(bp, np.float32)

    # fold 1x1 proj into the 3x3 conv:  G[h,co,ci,ky,kx] = sum_cm Wp[co,(h,cm)] Wv[(h,cm),ci,ky,kx]
    Wv5 = Wv.reshape(NH, DV, DV, 3, 3)
    Wp3 = Wp.reshape(DV, NH, DV)
    G = np.einsum('ohm,hmiyx->hoiyx', Wp3, Wv5).reshape(NH * DV, DV, 3, 3)

    WC = np.zeros((128, 4, 5, 128), np.float32)
    for hp in range(4):
        oc = np.arange(128) + hp * 128
        for j in range(5):
            ka, kb = _TAP_A[j], _TAP_B[j]
            WC[0:64, hp, j, :] = G[oc, :, ka[0], ka[1]].T
            if kb is not None:
                WC[64:128, hp, j, :] = G[oc, :, kb[0], kb[1]].T
    wc = np.ascontiguousarray(WC.reshape(128, 2560)).astype(ml_dtypes.bfloat16)

    wqk = np.zeros((2, 16, 2, 128, 128), np.float32)
    for i, Wmat in enumerate([Wq, Wk]):
        for m in range(16):
            for t in range(2):
                wqk[i, m, t] = Wmat[t * 128:(t + 1) * 128, m * 128:(m + 1) * 128]
    # flatten to [K=128 partitions, tile*128 + m-col] for the single preload
    wqk = np.ascontiguousarray(
        wqk.reshape(64, 128, 128).transpose(1, 0, 2).reshape(128, 8192)
    ).astype(ml_dtypes.bfloat16)

    bqk = np.zeros((128, 32), np.float32)
    bqk[:, 0:16] = bq.reshape(16, 128).T
    bqk[:, 16:32] = bk.reshape(16, 128).T

    ident = np.eye(16, dtype=np.float32)
    bias_total = Wp.reshape(DV, NH * DV) @ bv + bp
    return wc, wqk, bqk, ident, bias_total


def _get_graph():
    global _GRAPH
    if _GRAPH is None:
        _GRAPH = _build_graph()
    return _GRAPH


def kernel(v, k, q, prod_mask, Wq, bq, Wk, bk, Wv, bv, Wp, bp):
    global LAST_RESULTS
    nc = _get_graph()
    wc, wqk, bqk, ident, bias_total = _host_consts(Wq, bq, Wk, bk, Wv, bv, Wp, bp)

    v = np.ascontiguousarray(np.asarray(v, np.float32).reshape(B, TI, DV, HW))
    q = np.ascontiguousarray(np.asarray(q, np.float32))
    k = np.ascontiguousarray(np.asarray(k, np.float32))
    pm = np.ascontiguousarray(np.asarray(prod_mask, np.int32))

    in_maps = []
    for b in range(N_CORES):
        in_maps.append({
            "v": v[b], "q": q[b], "k": k[b], "mask": pm[b],
            "wc": wc, "wqk": wqk, "bqk": bqk, "ident": ident,
        })

    trace = bool(int(os.environ.get("KERNEL_TRACE", "0")))
    tmpdir = os.environ.get("KERNEL_TRACE_DIR") or None
    res = run_bass_kernel_spmd(nc, in_maps, core_ids=list(range(N_CORES)),
                               trace=trace, tmpdir=tmpdir)
    LAST_RESULTS = res

    out = np.stack([np.asarray(res.results[i]["out"]) for i in range(N_CORES)])
    out = out.astype(np.float32).reshape(B, TO, DV, H, W)
    out = out + bias_total[None, None, :, None, None]
    return np.ascontiguousarray(out)


# revision 26
# speedup vs baseline: 2.7879x; 1.9283x over previous
"""Trainium2 Bass kernel for nn_ConvMultiHeadAttention.

Strategy: data-parallel over batch B=8 across the 8 NeuronCores (no
collectives).  Per core (one batch element):

  1. q/k linear projections + per-head scaled-dot-product scores + masked
     softmax (bf16 matmuls, tiny).
  2. The 1x1 proj_concat conv is folded into the 3x3 value conv on the host
     (G_h = Wp_h @ Wv_h), so the conv directly produces the per-head
     projected values z[i, h, c, pos].
  3. 3x3 conv as 5 K=128 matmuls per (frame, 128-channel tile): taps
     are packed in pairs along K using column/row-shifted padded copies of
     the input frame held in SBUF partitions 0-63 / 64-127.
  4. PSUM -> SBUF bf16 eviction into a per-frame [128, 4*512] tile, then ONE
     scatter-DMA "transpose" per frame into the [(i,h) on partitions,
     (c, pos) free] layout (alternating sync/scalar DMA rings).
  5. Attention mix: 4x column-tiled K=128 matmuls (tile_position) contracting
     over (frame i, head h), 4 slices concurrently in the PE array, evicted
     into one big SBUF tile and written with ONE output DMA per chunk (bf16).

v2 changes vs baseline: all weights preloaded in single DMAs, v loaded via
gpsimd cast-DMAs straight to bf16, 16 scatter DMAs instead of 128, 2 output
DMAs instead of 128 -- the DMA rings and PE stay busy, HAM stays warm.
"""

import os
import numpy as np

import concourse.bass as bass
import concourse.bacc as bacc
import concourse.tile as tile
import concourse.mybir as mybir
from concourse.bass_utils import run_bass_kernel_spmd

NH, DQK, DV = 8, 256, 64
B, TI, TO, H, W = 8, 16, 16, 32, 32
HW = H * W           # 1024
PW = 34              # padded row width (32 + 2)
HR = 18              # padded rows resident per half-frame (16 + 2)
N_CORES = 8

F32 = mybir.dt.float32
BF16 = mybir.dt.bfloat16
I32 = mybir.dt.int32

# Tap pairing for the 5 conv matmuls (kernel indices (ky, kx) in 0..2).
# A-half = plain padded frame on partitions 0:64.
# j<3  : B-half (partitions 64:128) = frame shifted one column  -> covers kx+1
# j==3 : D-half (T2 tile)           = frame shifted one row     -> covers ky+1
# j==4 : single tap (2,2), B-half weights are zero.
_TAP_A = [(0, 0), (1, 0), (2, 0), (0, 2), (2, 2)]
_TAP_B = [(0, 1), (1, 1), (2, 1), (1, 2), None]

_GRAPH = None
LAST_RESULTS = None


def _build_graph():
    from contextlib import ExitStack

    nc = bacc.Bacc("TRN2", target_bir_lowering=False, debug=False,
                   num_devices=N_CORES)

    v_ap = nc.dram_tensor("v", [TI, DV, HW], F32, kind="ExternalInput").ap()
    q_ap = nc.dram_tensor("q", [TO, DQK], F32, kind="ExternalInput").ap()
    k_ap = nc.dram_tensor("k", [TI, DQK], F32, kind="ExternalInput").ap()
    m_ap = nc.dram_tensor("mask", [TO, TI], I32, kind="ExternalInput").ap()
    wc_ap = nc.dram_tensor("wc", [128, 2560], BF16, kind="ExternalInput").ap()
    wqk_ap = nc.dram_tensor("wqk", [128, 8192], BF16, kind="ExternalInput").ap()
    bqk_ap = nc.dram_tensor("bqk", [128, 32], F32, kind="ExternalInput").ap()
    id_ap = nc.dram_tensor("ident", [16, 16], F32, kind="ExternalInput").ap()
    out_ap = nc.dram_tensor("out", [TO, DV * HW], BF16, kind="ExternalOutput").ap()
    # DRAM bounce buffer for the z transpose, laid out in zT order:
    # [chunk, fr, h'=(hp,h2), c, n] so the readback is fully contiguous.
    zs_ap = nc.dram_tensor("zs", [2, 16, 8, 64, 512], BF16, kind="Internal").ap()

    AF = mybir.ActivationFunctionType
    OP = mybir.AluOpType

    with tile.TileContext(nc) as tc, ExitStack() as ctx:
        zevp = ctx.enter_context(tc.tile_pool(name="zevp", bufs=8))
        cps = ctx.enter_context(tc.tile_pool(name="cps", bufs=6, space="PSUM"))
        sps = ctx.enter_context(tc.tile_pool(name="sps", bufs=2, space="PSUM"))

        # persistent SBUF tensors (static allocations: no lifetime packing)
        def static(name, shape, dtype):
            return nc.alloc_sbuf_tensor(name, list(shape), dtype).ap()

        T1 = static("T1", [128, 16 * HR * PW], BF16)       # [A; B] per frame
        T2 = static("T2", [128, 16 * 16 * PW], BF16)       # [A; D] per frame
        vbf = static("vbf", [128, 16 * HW], BF16)          # frames duplicated 2x
        wc = static("wc_sb", [128, 2560], BF16)
        wqk = static("wqk_sb", [128, 8192], BF16)
        zT = static("zT", [128, 64 * 512], BF16)
        att = static("att_sb", [128, 16 * 512], BF16)
        qk = static("qk_sb", [16, 512], F32)
        qkT = static("qkT", [128, 64], BF16)
        pqT = static("pqT", [128, 256], BF16)
        pkT = static("pkT", [128, 256], BF16)
        wflat = static("wflat", [128, 32], BF16)
        id_sb = static("id_sb", [16, 16], F32)
        bqk = static("bqk_sb", [128, 32], F32)
        mi = static("mi_sb", [16, 16], I32)
        mb = static("mb", [16, 16], F32)
        mbig = static("mbig", [16, 128], F32)
        s1 = static("s1", [16, 128], F32)
        s2 = static("s2", [16, 128], F32)
        s3 = static("s3", [16, 128], F32)
        s4 = static("s4", [16, 128], F32)
        rmax = static("rmax", [16, 8], F32)
        rsum = static("rsum", [16, 8], F32)
        rinv = static("rinv", [16, 8], F32)

        t1v = T1[:].rearrange("p (f r c) -> p f r c", r=HR, c=PW)
        t2v = T2[:].rearrange("p (f r c) -> p f r c", r=16, c=PW)
        vbfv = vbf[:].rearrange("p (f y x) -> p f y x", y=H, x=W)

        # ---------- input / constant loads ----------
        # sync ring: small qk-phase inputs first, then the big weight blocks
        nc.sync.dma_start(qk[:, 0:256], q_ap[:, :])
        nc.sync.dma_start(qk[:, 256:512], k_ap[:, :])
        nc.sync.dma_start(id_sb[:], id_ap[:, :])
        nc.sync.dma_start(mi[:], m_ap[:, :])
        nc.sync.dma_start(bqk[:], bqk_ap[:, :])
        nc.sync.dma_start(wqk[:], wqk_ap[:, :])
        nc.sync.dma_start(wc[:], wc_ap[:, :])

        # v frames: gpsimd (SWDGE) DMAs cast f32 -> bf16 on the fly and land
        # the frames duplicated on partitions 0:64 / 64:128.  Split into 4
        # DMAs so staging of frames 0-7 can start while 8-15 still load.
        vbf_h0 = vbf[0:64].rearrange("p (f n) -> p f n", n=HW)
        vbf_h1 = vbf[64:128].rearrange("p (f n) -> p f n", n=HW)
        for lo, hi in ((0, 8), (8, 16)):
            src = v_ap[lo:hi].rearrange("f c n -> c f n")
            nc.gpsimd.dma_start(vbf_h0[:, lo:hi, :], src)
            nc.gpsimd.dma_start(vbf_h1[:, lo:hi, :], src)

        # zero-init the padded v staging tensors (pad cells must be 0)
        nc.vector.memset(T1[:].bitcast(F32), 0.0)
        nc.vector.memset(T2[:].bitcast(F32), 0.0)

        # ---------- scores / softmax phase ----------
        # All PSUM tiles in the sps pool share one tag/shape (a full bank)
        # so the pool costs exactly 2 banks; small users slice it.
        def sps_tile():
            return sps.tile([128, 512], F32, name="spst", tag="spst")

        # qT / kT tiles via PE transpose: qkT cols [q-t0 | q-t1 | k-t0 | k-t1]
        for j in range(4):
            half, t = j // 2, j % 2
            ps = sps_tile()
            nc.tensor.transpose(
                ps[:, 0:16], qk[0:16, half * 256 + t * 128: half * 256 + (t + 1) * 128],
                id_sb[:])
            nc.vector.tensor_copy(qkT[:, j * 16:(j + 1) * 16], ps[:, 0:16])

        # pqT / pkT: per m-tile of 128 (h,d)-rows, contract d' over 2 K-tiles
        for src in range(2):
            dst = pqT if src == 0 else pkT
            for m in range(16):
                ps = sps_tile()
                for t in range(2):
                    ti = (src * 32 + m * 2 + t) * 128
                    nc.tensor.matmul(
                        ps[:, 0:16], wqk[:, ti:ti + 128],
                        qkT[:, (src * 2 + t) * 16:(src * 2 + t + 1) * 16],
                        start=(t == 0), stop=(t == 1))
                nc.vector.tensor_scalar_add(
                    dst[:, m * 16:(m + 1) * 16], ps[:, 0:16],
                    bqk[:, src * 16 + m: src * 16 + m + 1])

        # scores[o, (h,i)]: per head contract over d (2 m-tiles)
        sc_t = sps_tile()
        sc = sc_t[0:16, 0:128]
        for h in range(8):
            for t in range(2):
                sl = slice((2 * h + t) * 16, (2 * h + t + 1) * 16)
                nc.tensor.matmul(sc[:, h * 16:(h + 1) * 16], pqT[:, sl], pkT[:, sl],
                                 start=(t == 0), stop=(t == 1))

        # masked softmax over i within each head block
        nc.scalar.activation(s1[:], sc, AF.Copy, scale=1.0 / 16.0)
        nc.vector.tensor_copy(mb[:], mi[:])
        nc.vector.tensor_scalar(mb[:], mb[:], 1.0e10, -1.0e10, OP.mult, OP.add)
        for h in range(8):
            nc.vector.tensor_copy(mbig[:, h * 16:(h + 1) * 16], mb[:])
        nc.vector.tensor_tensor(s2[:], s1[:], mbig[:], op=OP.add)
        nc.vector.reduce_max(rmax[:], s2[:].rearrange("p (h i) -> p h i", i=16),
                             axis=mybir.AxisListType.X)
        for h in range(8):
            nc.vector.tensor_scalar(s3[:, h * 16:(h + 1) * 16],
                                    s2[:, h * 16:(h + 1) * 16],
                                    rmax[:, h:h + 1], None, OP.subtract)
        nc.scalar.activation(s4[:], s3[:], AF.Exp)
        nc.vector.reduce_sum(rsum[:], s4[:].rearrange("p (h i) -> p h i", i=16),
                             axis=mybir.AxisListType.X)
        nc.vector.reciprocal(rinv[:], rsum[:])
        # write normalized weights interleaved: s3 free index = i*8 + h, so the
        # transpose below yields wflat partitions p = i*8 + h (zT layout).
        for h in range(8):
            nc.vector.tensor_scalar(s3[:, h::8],
                                    s4[:, h * 16:(h + 1) * 16],
                                    rinv[:, h:h + 1], None, OP.mult)
        # wflat cols 16:32 stay zero so the 32-wide col-tiled mix matmuls
        # write fully-defined PSUM ranges (rows 16:32 produce zeros).
        nc.vector.memset(wflat[:].bitcast(F32), 0.0)
        wt_ps = sps_tile()
        nc.tensor.transpose(wt_ps[:, 0:16], s3[:], id_sb[:])
        nc.vector.tensor_copy(wflat[:, 0:16], wt_ps[:, 0:16])

        # ---------- conv + transpose + mix, per 16-row chunk ----------
        attv = att[:].rearrange("p (g n) -> p g n", n=512)

        for chunk in range(2):
            r0 = chunk * 16
            if chunk == 1:
                # row 17 of each T1 frame was data for half 0, pad for half 1
                nc.vector.memset(t1v[:, :, 17, :].bitcast(F32), 0.0)

            for fr in range(16):
                # staging: T1 (A/B halves) on vector, T2 (A/D halves) on gpsimd
                if chunk == 0:
                    # A/B rows rr 1..17  <- v rows 0..16 ; rr 0 stays zero (pad)
                    nc.vector.tensor_copy(t1v[0:64, fr, 1:18, 1:33], vbfv[0:64, fr, 0:17, :])
                    nc.vector.tensor_copy(t1v[64:128, fr, 1:18, 0:32], vbfv[64:128, fr, 0:17, :])
                    # T2: A rows rr 1..15 <- v rows 0..14 (rr 0 = pad row)
                    nc.vector.tensor_copy(t2v[0:64, fr, 1:16, 1:33], vbfv[0:64, fr, 0:15, :])
                else:
                    # A/B rows rr 0..16 <- v rows 15..31 ; rr 17 zeroed above
                    nc.vector.tensor_copy(t1v[0:64, fr, 0:17, 1:33], vbfv[0:64, fr, 15:32, :])
                    nc.vector.tensor_copy(t1v[64:128, fr, 0:17, 0:32], vbfv[64:128, fr, 15:32, :])
                    nc.vector.tensor_copy(t2v[0:64, fr, 0:16, 1:33], vbfv[0:64, fr, 15:31, :])
                # T2 D-half rows rr 0..15 <- v rows r0..r0+15 (always valid)
                nc.vector.tensor_copy(t2v[64:128, fr, 0:16, 1:33], vbfv[64:128, fr, r0:r0 + 16, :])

            for fr in range(16):
                zev = zevp.tile([128, 4 * 512], BF16, name="zev", tag="zev")
                for hp in range(4):
                    ps = cps.tile([128, 16, 32], F32, name="cpst", tag="cpst")
                    for j in range(5):
                        if j < 3:
                            rhs = t1v[:, fr, j:j + 16, 0:32]
                        elif j == 3:
                            rhs = t2v[:, fr, 0:16, 2:34]
                        else:
                            rhs = t1v[:, fr, 2:18, 2:34]
                        lhsT = wc[:, (hp * 5 + j) * 128:(hp * 5 + j + 1) * 128]
                        nc.tensor.matmul(ps[:], lhsT, rhs,
                                         start=(j == 0), stop=(j == 4))
                    # evict PSUM -> bf16 quarter of the per-frame tile (ACT;
                    # DVE is busy with staging + mix evictions)
                    nc.scalar.copy(zev[:, hp * 512:(hp + 1) * 512], ps[:])
                # transpose hop 1: per-frame transposing write to DRAM in zT
                # order (src partition (h2,c) -> dst dim (h2 c), uniform
                # stride after merging, so both APs stay 3-dim)
                dst = zs_ap[chunk, fr].rearrange("(hp h2) c n -> (h2 c) hp n", h2=2)
                eng = (nc.sync, nc.scalar)[fr % 2]
                eng.dma_start(dst, zev[:].rearrange("p (hp n) -> p hp n", n=512))
                # transpose hop 2: after each 8-frame half lands, a fully
                # contiguous readback into zT rows [g*64, (g+1)*64)
                if fr % 8 == 7:
                    g = fr // 8
                    src = zs_ap[chunk, 8 * g:8 * g + 8].rearrange(
                        "f e c n -> (f e) (c n)")
                    eng = (nc.sync, nc.scalar)[g % 2]
                    eng.dma_start(zT[64 * g:64 * (g + 1)], src)

            # mix: 4 column-tiled K=128 matmuls at once (slices s=4g+jj),
            # output rows o live at PSUM partitions 32*jj .. 32*jj+16
            for g in range(16):
                mp = sps_tile()
                for jj in range(4):
                    s = g * 4 + jj
                    nc.tensor.matmul(mp[32 * jj:32 * jj + 32, :], wflat[:],
                                     zT[:, s * 512:(s + 1) * 512],
                                     start=True, stop=True,
                                     tile_position=(0, 32 * jj))
                nc.vector.tensor_copy(attv[:, g, :], mp[:])

            # 4 output DMAs per chunk (bf16, one per col-group); host casts
            # to f32 and adds the bias.
            outv = out_ap.rearrange("o (g jj pos) -> jj o g pos", jj=4, pos=1024)
            for jj in range(4):
                src = att[32 * jj:32 * jj + 16].rearrange("p (g n) -> p g n", n=512)
                dst = outv[jj, :, :, chunk * 512:(chunk + 1) * 512]
                eng = nc.scalar if jj % 2 == 0 else nc.sync
                eng.dma_start(dst, src)

    nc.compile()
    return nc


def _host_consts(Wq, bq, Wk, bk, Wv, bv, Wp, bp):
    import ml_dtypes

    Wq = np.asarray(Wq, np.float32)
    Wk = np.asarray(Wk, np.float32)
    Wv = np.asarray(Wv, np.float32)
    Wp = np.asarray(Wp, np.float32)
    bq = np.asarray(bq, np.float32)
    bk = np.asarray(bk, np.float32)
    bv = np.asarray(bv, np.float32)
    bp = np.asarray(bp, np.float32)

    # fold 1x1 proj into the 3x3 conv:  G[h,co,ci,ky,kx] = sum_cm Wp[co,(h,cm)] Wv[(h,cm),ci,ky,kx]
    Wv5 = Wv.reshape(NH, DV, DV, 3, 3)
    Wp3 = Wp.reshape(DV, NH, DV)
    G = np.einsum('ohm,hmiyx->hoiyx', Wp3, Wv5).reshape(NH * DV, DV, 3, 3)

    WC = np.zeros((128, 4, 5, 128), np.float32)
    for hp in range(4):
        oc = np.arange(128) + hp * 128
        for j in range(5):
            ka, kb = _TAP_A[j], _TAP_B[j]
            WC[0:64, hp, j, :] = G[oc, :, ka[0], ka[1]].T
            if kb is not None:
                WC[64:128, hp, j, :] = G[oc, :, kb[0], kb[1]].T
    wc = np.ascontiguousarray(WC.reshape(128, 2560)).astype(ml_dtypes.bfloat16)

    wqk = np.zeros((2, 16, 2, 128, 128), np.float32)
    for i, Wmat in enumerate([Wq, Wk]):
        for m in range(16):
            for t in range(2):
                wqk[i, m, t] = Wmat[t * 128:(t + 1) * 128, m * 128:(m + 1) * 128]
    # flatten to [K=128 partitions, tile*128 + m-col] for the single preload
    wqk = np.ascontiguousarray(
        wqk.reshape(64, 128, 128).transpose(1, 0, 2).reshape(128, 8192)
    ).astype(ml_dtypes.bfloat16)

    bqk = np.zeros((128, 32), np.float32)
    bqk[:, 0:16] = bq.reshape(16, 128).T
    bqk[:, 16:32] = bk.reshape(16, 128).T

    ident = np.eye(16, dtype=np.float32)
    bias_total = Wp.reshape(DV, NH * DV) @ bv + bp
    return wc, wqk, bqk, ident, bias_total


def _get_graph():
    global _GRAPH
    if _GRAPH is None:
        _GRAPH = _build_graph()
    return _GRAPH


def kernel(v, k, q, prod_mask, Wq, bq, Wk, bk, Wv, bv, Wp, bp):
    global LAST_RESULTS
    nc = _get_graph()
    wc, wqk, bqk, ident, bias_total = _host_consts(Wq, bq, Wk, bk, Wv, bv, Wp, bp)

    v = np.ascontiguousarray(np.asarray(v, np.float32).reshape(B, TI, DV, HW))
    q = np.ascontiguousarray(np.asarray(q, np.float32))
    k = np.ascontiguousarray(np.asarray(k, np.float32))
    pm = np.ascontiguousarray(np.asarray(prod_mask, np.int32))

    in_maps = []
    for b in range(N_CORES):
        in_maps.append({
            "v": v[b], "q": q[b], "k": k[b], "mask": pm[b],
            "wc": wc, "wqk": wqk, "bqk": bqk, "ident": ident,
        })

    trace = bool(int(os.environ.get("KERNEL_TRACE", "0")))
    tmpdir = os.environ.get("KERNEL_TRACE_DIR") or None
    res = run_bass_kernel_spmd(nc, in_maps, core_ids=list(range(N_CORES)),
                               trace=trace, tmpdir=tmpdir)
    LAST_RESULTS = res

    out = np.stack([np.asarray(res.results[i]["out"]) for i in range(N_CORES)])
    out = out.astype(np.float32).reshape(B, TO, DV, H, W)
    out = out + bias_total[None, None, :, None, None]
    return np.ascontiguousarray(out)
